# revision 22
# baseline (speedup 1.0000x reference)
"""NetVLAD Trainium2 kernel — data-parallel over N across 8 cores.

v2: bf16 PE datapath + fp16 softmax chain + ln/exp-based rsqrt (single
activation table), host-side bf16 upload (halves DMA), merged
logits+transpose matmul, software-pipelined vlad.

Per core: 4 images [C=128, P=4096], chunks of 1024 px (8 tiles of 128).
  PE per tile:  psum[px, 0:64]=logits_raw, [64:192]=xT  via one matmul
                xb_t.T @ [wT | I] (bf16);  ssq via xsqb_t.T @ ones.
  softmax (k in free dim):  inv_n = exp(-.5 ln ssq) [ACT], lu = raw*inv_n
  [DVE fp16], ll = lu + b [DVE fp16], negm = -max_k [DVE], per-tile
  ee = Exp(ll + negm_t) with accum -> sumexp [ACT, bf16 out],
  r = inv_n/sumexp [DVE bf16], aa = ee*r [DVE bf16].
  gpsimd evicts xT psum -> xTs bf16 [px, (8,129)], col 128 = n.
  PE: psV[56, 0:129] += aa_t[:, :56].T @ xTs_t  (bf16, accum over image).
Tail per image in the psV bank: vk = term1 - s*cen, PE transpose,
intra/global norms via Square-accum + ln/exp, transpose back, DMA out.
"""

import sys

for _p in ("/opt/trn_rl_repo",):
    if _p not in sys.path:
        sys.path.insert(0, _p)

import numpy as np

NIMG = 4      # images per core
C = 128
K = 64
KE = 56
P = 4096
TPC = 8       # 128-px tiles per chunk
CH = TPC * 128
NCH = P // CH           # 4 chunks per image
NT = NIMG * NCH         # 16 chunks per core

_cache = {}


def _build():
    import concourse.bass as bass
    import concourse.mybir as mybir
    from concourse import bacc, tile

    f32 = mybir.dt.float32
    f16 = mybir.dt.float16
    bf16 = mybir.dt.bfloat16
    Alu = mybir.AluOpType
    Act = mybir.ActivationFunctionType
    AxX = mybir.AxisListType.X

    nc = bacc.Bacc()
    x_in = nc.declare_dram_parameter("xb", [NIMG, C, P], bf16, isOutput=False)
    # cstb bf16 [C, 193]: 0:64 wT | 64:192 ident | 192 ones
    cb_in = nc.declare_dram_parameter("cstb", [C, 193], bf16, isOutput=False)
    # csth fp32 [C, 512]: conv_b tiled 8x
    ch_in = nc.declare_dram_parameter("csth", [C, 512], f32, isOutput=False)
    # cstf fp32 [C, 400]: 0:128 ident | 128:256 cen(rows 0:56) | 256 ones-col
    # | 258:386 ones-row (row 0) | 392:400 = -0.5 block
    cf_in = nc.declare_dram_parameter("cstf", [C, 400], f32, isOutput=False)
    out_ext = nc.declare_dram_parameter("out", [NIMG, KE, C], f32, isOutput=True)

    with tile.TileContext(nc) as tc:
        with (
            tc.tile_pool(name="const", bufs=1) as cpool,
            tc.tile_pool(name="xin", bufs=3) as xpool,
            tc.tile_pool(name="work", bufs=2) as wpool,
            tc.tile_pool(name="stats", bufs=2) as spool,
            tc.tile_pool(name="fin", bufs=2) as fpool,
            tc.tile_pool(name="psL", bufs=2, space="PSUM") as pL,
            tc.tile_pool(name="psT", bufs=2, space="PSUM") as pT,
            tc.tile_pool(name="psS", bufs=1, space="PSUM") as pS,
            tc.tile_pool(name="psV", bufs=1, space="PSUM") as pV,
        ):
            cstb = cpool.tile([C, 193], bf16, tag="cstb")
            csth = cpool.tile([C, 512], f32, tag="csth")
            cstf = cpool.tile([C, 400], f32, tag="cstf")
            nc.sync.dma_start(cstb[:], cb_in[:])
            nc.sync.dma_start(csth[:], ch_in[:])
            nc.sync.dma_start(cstf[:], cf_in[:])
            wTb = cstb[:, 0:K]
            identb = cstb[:, K:K + C]
            onesb = cstb[:, 192:193]
            b8h = csth[:]                 # fp16 bias, tiled 8x
            identf = cstf[:, 0:128]
            cen = cstf[0:KE, 128:256]
            onesf = cstf[:, 256:257]
            onesrow = cstf[0:1, 258:386]
            neghalf = cstf[:, 392:400]

            state = {}

            def emit_dma(t):
                if t >= NT or t in state.setdefault("dma", {}):
                    return
                img, ch = divmod(t, NCH)
                xb = xpool.tile([C, CH], bf16, tag="x", name="xb")
                nc.sync.dma_start(xb[:], x_in[img, :, ch * CH:(ch + 1) * CH])
                state["dma"][t] = xb

            def emit_f2(t):
                """n-pipeline of chunk t (two chunks ahead): squares, ssq
                matmuls, rsqrt.  The slow gpsimd pow is far off-chain."""
                if t >= NT:
                    return
                emit_dma(t)
                emit_dma(t + 1)
                xb = state["dma"][t]
                xsq = wpool.tile([C, CH], bf16, tag="xsq", name="xsq")
                nc.vector.tensor_mul(xsq[:], xb[:], xb[:])
                pss = pS.tile([C, 8], f32, tag="S", name="pss")
                for j in range(TPC):
                    nc.tensor.matmul(pss[:, j:j + 1],
                                     xsq[:, j * 128:(j + 1) * 128], onesb,
                                     start=True, stop=True)
                ssqs = spool.tile([C, 8], f32, tag="ssqs", name="ssqs")
                nc.vector.tensor_copy(ssqs[:], pss[:])
                invc = spool.tile([C, 8], f32, tag="invc", name="invc")
                nc.gpsimd.tensor_tensor(invc[:], ssqs[:], neghalf, Alu.pow)
                state[("f2", t)] = (ssqs, invc)

            def emit_f1(t):
                """Logits + transpose matmuls of chunk t (one chunk ahead)."""
                if t >= NT:
                    return
                xb = state["dma"].pop(t)
                psl = pL.tile([C, TPC * K], f32, tag="L", name="psl")
                pst = pT.tile([C, TPC * 128], f32, tag="T", name="pst")
                for j in range(TPC):
                    xt = xb[:, j * 128:(j + 1) * 128]
                    nc.tensor.matmul(psl[:, j * K:(j + 1) * K], xt, wTb,
                                     start=True, stop=True)
                    nc.tensor.matmul(pst[:, j * 128:(j + 1) * 128], xt,
                                     identb, start=True, stop=True)
                state[("f1", t)] = (psl, pst)

            def emit_back(t):
                """Chain tail of chunk t: softmax, scaled eviction, vlad."""
                img, ch = divmod(t, NCH)
                ssqs, invc = state.pop(("f2", t))
                psl, pst = state.pop(("f1", t))
                psumL = psl[:].rearrange("p (t k) -> p t k", k=K)

                l3 = lambda tl, q: tl[:].rearrange("p (t k) -> p t k", k=q)
                # lu = raw * inv_n
                lu = wpool.tile([C, TPC * K], f32, tag="lu")
                nc.vector.tensor_tensor(
                    l3(lu, K), psumL, invc[:].broadcast_to([C, TPC, K]),
                    Alu.mult)
                # ll = lu + b  (gpsimd)
                ll = wpool.tile([C, TPC * K], f32, tag="ll")
                nc.gpsimd.tensor_tensor(ll[:], lu[:], b8h, Alu.add)
                # negm = -max_k ll
                negm = spool.tile([C, 8], f32, tag="negm")
                nc.vector.tensor_reduce(negm[:], l3(ll, K), axis=AxX,
                                        op=Alu.max, negate=True)
                # ee_j = exp(ll_j - m_j) per tile (bias folds the max-sub)
                ee = wpool.tile([C, TPC * K], bf16, tag="ee")
                for j in range(TPC):
                    nc.scalar.activation(ee[:, j * K:(j + 1) * K],
                                         ll[:, j * K:(j + 1) * K], Act.Exp,
                                         bias=negm[:, j:j + 1])
                scol = spool.tile([C, 8], f32, tag="scol")
                nc.vector.tensor_reduce(scol[:], l3(ee, K), axis=AxX,
                                        op=Alu.add)
                # r = inv_n / sumexp; vlad pixel scale rides on the eviction
                gcol = spool.tile([C, 8], f32, tag="gcol")
                nc.vector.reciprocal(gcol[:], scol[:])
                rcol = spool.tile([C, 8], f32, tag="rcol")
                nc.vector.tensor_tensor(rcol[:], invc[:], gcol[:], Alu.mult)
                # col 128 of each slab: n*r  (n = ssq*inv_n)
                svec = spool.tile([C, 8], f32, tag="svec")
                nc.vector.tensor_tensor(svec[:], ssqs[:], invc[:], Alu.mult)
                nrv = spool.tile([C, 8], f32, tag="nrv")
                nc.vector.tensor_tensor(nrv[:], svec[:], rcol[:], Alu.mult)

                xTs = wpool.tile([C, TPC * 129], bf16, tag="xTs", name="xTs")
                xv = xTs[:].rearrange("p (t q) -> p t q", q=129)
                s3 = lambda tl: tl[:].rearrange("p (t o) -> p t o", o=1)
                nc.scalar.activation(xv[:, :, 128:129], s3(nrv), Act.Copy)

                if ch == 0:
                    state["psV"] = pV.tile([C, 512], f32, tag="psV",
                                           name="psv")
                psv = state["psV"]
                pv_ = pst[:].rearrange("p (t q) -> p t q", q=128)
                for j in range(TPC):
                    # scaled eviction: xTs_j = xT_j * r_j  (ACT, per tile)
                    nc.scalar.activation(xv[:, j, 0:128], pv_[:, j, :],
                                         Act.Copy, scale=rcol[:, j:j + 1])
                    nc.tensor.matmul(psv[0:KE, 0:129],
                                     ee[:, j * K:j * K + KE],
                                     xTs[:, j * 129:(j + 1) * 129],
                                     start=(ch == 0 and j == 0),
                                     stop=(ch == NCH - 1 and j == TPC - 1))
                if ch == NCH - 1:
                    emit_tail(img, psv)

            def emit_tail(img, psv):
                # vk = term1 - s*cen  [56, 128] fp32
                negs = spool.tile([KE, 1], f32, tag="negs")
                nc.vector.tensor_scalar_mul(negs[:], psv[0:KE, 128:129], -1.0)
                vk = fpool.tile([KE, C], f32, tag="vk")
                nc.vector.scalar_tensor_tensor(vk[:], cen, negs[:],
                                               psv[0:KE, 0:C],
                                               Alu.mult, Alu.add)
                # transpose -> [c, k] into the same psV bank (cols 160:216)
                nc.tensor.matmul(psv[:, 160:160 + KE], vk[:],
                                 identf[0:KE, 0:KE],
                                 is_transpose=True, start=True, stop=True)
                vkT = psv[:, 160:160 + KE]
                trash = fpool.tile([C, KE], bf16, tag="trash")
                ssqk = spool.tile([C, 1], f32, tag="ssqk")
                nc.scalar.activation(trash[:], vkT, Act.Square,
                                     accum_out=ssqk[:])
                ssqc = spool.tile([C, 1], f32, tag="ssqc")
                nc.vector.tensor_scalar_max(ssqc[:], ssqk[:], 1e-24)
                invk = spool.tile([C, 1], f32, tag="invk")
                nc.gpsimd.tensor_tensor(invk[:], ssqc[:], neghalf[:, 0:1],
                                        Alu.pow)
                # q = ssqk * invk^2  (per-partition contribution to ||.||_F^2)
                iv2 = spool.tile([C, 1], f32, tag="iv2")
                nc.vector.tensor_tensor(iv2[:], invk[:], invk[:], Alu.mult)
                qv = spool.tile([C, 1], f32, tag="qv")
                nc.vector.tensor_tensor(qv[:], ssqc[:], iv2[:], Alu.mult)
                # tot = sum_c q  via PE (fp32 tiny)
                nc.tensor.matmul(psv[0:1, 216:217], qv[:], onesf,
                                 start=True, stop=True)
                tot = spool.tile([1, 1], f32, tag="tot")
                nc.vector.tensor_scalar_max(tot[:], psv[0:1, 216:217], 1e-24)
                fv = spool.tile([1, 1], f32, tag="fv")
                nc.gpsimd.tensor_tensor(fv[:], tot[:], neghalf[0:1, 0:1],
                                        Alu.pow)
                # broadcast fv to all partitions, comb = invk * fv
                nc.tensor.matmul(psv[:, 218:219], onesrow, fv[:],
                                 start=True, stop=True)
                comb = spool.tile([C, 1], f32, tag="comb")
                nc.vector.tensor_tensor(comb[:], invk[:], psv[:, 218:219],
                                        Alu.mult)
                obT = fpool.tile([C, KE], f32, tag="obT")
                nc.vector.tensor_scalar(obT[:], vkT, comb[:], None, Alu.mult)
                # transpose back -> [k, c] (cols 256:384), evict, DMA out
                nc.tensor.matmul(psv[0:KE, 256:384], obT[:], identf,
                                 is_transpose=True, start=True, stop=True)
                ob = fpool.tile([KE, C], f32, tag="ob")
                nc.scalar.activation(ob[:], psv[0:KE, 256:384], Act.Copy)
                nc.sync.dma_start(out_ext[img], ob[:])

            emit_f2(0)
            emit_f2(1)
            emit_f1(0)
            for t in range(NT):
                emit_f2(t + 2)
                emit_f1(t + 1)
                emit_back(t)

    nc.compile()
    return nc


def _get_nc():
    if "nc" not in _cache:
        _cache["nc"] = _build()
    return _cache["nc"]


def _make_in_maps(inputs):
    import ml_dtypes

    x = np.asarray(inputs["x"], dtype=np.float32)
    conv_w = np.asarray(inputs["conv_w"], dtype=np.float32)
    conv_b = np.asarray(inputs["conv_b"], dtype=np.float32)
    centroids = np.asarray(inputs["centroids"], dtype=np.float32)

    N = x.shape[0]
    n_cores = 8
    per = N // n_cores
    assert per == NIMG

    xb = x.reshape(N, C, P).astype(ml_dtypes.bfloat16)

    cstb = np.zeros((C, 193), dtype=ml_dtypes.bfloat16)
    cstb[:, 0:K] = conv_w.T.astype(ml_dtypes.bfloat16)
    cstb[:, K:K + C] = np.eye(C, dtype=np.float32)
    cstb[:, 192] = 1.0

    csth = np.tile(conv_b.astype(np.float32), TPC)[None, :].repeat(C, axis=0)
    csth = np.ascontiguousarray(csth)

    cstf = np.zeros((C, 400), dtype=np.float32)
    cstf[:, 0:C] = np.eye(C, dtype=np.float32)
    cstf[0:KE, C:C + C] = centroids[:KE]
    cstf[:, 256] = 1.0
    cstf[0, 258:386] = 1.0
    cstf[:, 392:400] = -0.5

    in_maps = []
    for i in range(n_cores):
        in_maps.append({
            "xb": np.ascontiguousarray(xb[i * per:(i + 1) * per]),
            "cstb": cstb,
            "csth": csth,
            "cstf": cstf,
        })
    return in_maps


def kernel(x, conv_w, conv_b, centroids):
    from concourse.bass_utils import run_bass_kernel_spmd

    in_maps = _make_in_maps(
        {"x": x, "conv_w": conv_w, "conv_b": conv_b, "centroids": centroids}
    )
    nc = _get_nc()
    res = run_bass_kernel_spmd(nc, in_maps, list(range(8)))
    outs = [np.asarray(r["out"]).reshape(NIMG, KE * C) for r in res.results]
    return np.concatenate(outs, axis=0)


if __name__ == "__main__":
    rng = np.random.default_rng(0)
    x = rng.standard_normal((32, C, 64, 64), dtype=np.float32)
    w = rng.standard_normal((K, C), dtype=np.float32)
    b = rng.standard_normal((K,), dtype=np.float32)
    c = rng.random((K, C), dtype=np.float32)
    out = kernel(x=x, conv_w=w, conv_b=b, centroids=c)
    print(out.shape, out.dtype)


# revision 23
# speedup vs baseline: 1.1292x; 1.1292x over previous
"""NetVLAD Trainium2 kernel — data-parallel over N across 8 cores.

v2: bf16 PE datapath + fp16 softmax chain + ln/exp-based rsqrt (single
activation table), host-side bf16 upload (halves DMA), merged
logits+transpose matmul, software-pipelined vlad.

Per core: 4 images [C=128, P=4096], chunks of 1024 px (8 tiles of 128).
  PE per tile:  psum[px, 0:64]=logits_raw, [64:192]=xT  via one matmul
                xb_t.T @ [wT | I] (bf16);  ssq via xsqb_t.T @ ones.
  softmax (k in free dim):  inv_n = exp(-.5 ln ssq) [ACT], lu = raw*inv_n
  [DVE fp16], ll = lu + b [DVE fp16], negm = -max_k [DVE], per-tile
  ee = Exp(ll + negm_t) with accum -> sumexp [ACT, bf16 out],
  r = inv_n/sumexp [DVE bf16], aa = ee*r [DVE bf16].
  gpsimd evicts xT psum -> xTs bf16 [px, (8,129)], col 128 = n.
  PE: psV[56, 0:129] += aa_t[:, :56].T @ xTs_t  (bf16, accum over image).
Tail per image in the psV bank: vk = term1 - s*cen, PE transpose,
intra/global norms via Square-accum + ln/exp, transpose back, DMA out.
"""

import sys

for _p in ("/opt/trn_rl_repo",):
    if _p not in sys.path:
        sys.path.insert(0, _p)

import numpy as np

NIMG = 4      # images per core
C = 128
K = 64
KE = 56
P = 4096
TPC = 8       # 128-px tiles per chunk
CH = TPC * 128
NCH = P // CH           # 4 chunks per image
NT = NIMG * NCH         # 16 chunks per core

_cache = {}


def _build():
    import concourse.bass as bass
    import concourse.mybir as mybir
    from concourse import bacc, tile

    f32 = mybir.dt.float32
    f16 = mybir.dt.float16
    bf16 = mybir.dt.bfloat16
    Alu = mybir.AluOpType
    Act = mybir.ActivationFunctionType
    AxX = mybir.AxisListType.X

    nc = bacc.Bacc()
    x_in = nc.declare_dram_parameter("xb", [NIMG, C, P], bf16, isOutput=False)
    # cstb bf16 [C, 193]: 0:64 wT | 64:192 ident | 192 ones
    cb_in = nc.declare_dram_parameter("cstb", [C, 193], bf16, isOutput=False)
    # csth fp32 [C, 512]: conv_b tiled 8x
    ch_in = nc.declare_dram_parameter("csth", [C, 512], f32, isOutput=False)
    # cstf fp32 [C, 400]: 0:128 ident | 128:256 cen(rows 0:56) | 256 ones-col
    # | 258:386 ones-row (row 0) | 392:400 = -0.5 block
    cf_in = nc.declare_dram_parameter("cstf", [C, 400], f32, isOutput=False)
    out_ext = nc.declare_dram_parameter("out", [NIMG, KE, C], f32, isOutput=True)

    with tile.TileContext(nc) as tc:
        with (
            tc.tile_pool(name="const", bufs=1) as cpool,
            tc.tile_pool(name="xin", bufs=3) as xpool,
            tc.tile_pool(name="work", bufs=2) as wpool,
            tc.tile_pool(name="stats", bufs=2) as spool,
            tc.tile_pool(name="fin", bufs=2) as fpool,
            tc.tile_pool(name="psL", bufs=2, space="PSUM") as pL,
            tc.tile_pool(name="psT", bufs=2, space="PSUM") as pT,
            tc.tile_pool(name="psS", bufs=1, space="PSUM") as pS,
            tc.tile_pool(name="psV", bufs=1, space="PSUM") as pV,
        ):
            cstb = cpool.tile([C, 193], bf16, tag="cstb")
            csth = cpool.tile([C, 512], f32, tag="csth")
            cstf = cpool.tile([C, 400], f32, tag="cstf")
            nc.sync.dma_start(cstb[:], cb_in[:])
            nc.sync.dma_start(csth[:], ch_in[:])
            nc.sync.dma_start(cstf[:], cf_in[:])
            wTb = cstb[:, 0:K]
            identb = cstb[:, K:K + C]
            onesb = cstb[:, 192:193]
            b8h = csth[:]                 # fp16 bias, tiled 8x
            identf = cstf[:, 0:128]
            cen = cstf[0:KE, 128:256]
            onesf = cstf[:, 256:257]
            onesrow = cstf[0:1, 258:386]
            neghalf = cstf[:, 392:400]

            state = {}

            def emit_dma(t):
                if t >= NT or t in state.setdefault("dma", {}):
                    return
                img, ch = divmod(t, NCH)
                xb = xpool.tile([C, CH], bf16, tag="x", name="xb")
                nc.sync.dma_start(xb[:], x_in[img, :, ch * CH:(ch + 1) * CH])
                state["dma"][t] = xb

            def emit_f2(t):
                """n-pipeline of chunk t (two chunks ahead): squares, ssq
                matmuls, rsqrt.  The slow gpsimd pow is far off-chain."""
                if t >= NT:
                    return
                emit_dma(t)
                emit_dma(t + 1)
                xb = state["dma"][t]
                xsq = wpool.tile([C, CH], bf16, tag="xsq", name="xsq")
                nc.scalar.activation(xsq[:], xb[:], Act.Square)
                pss = pS.tile([C, 8], f32, tag="S", name="pss")
                for j in range(TPC):
                    nc.tensor.matmul(pss[:, j:j + 1],
                                     xsq[:, j * 128:(j + 1) * 128], onesb,
                                     start=True, stop=True)
                ssqs = spool.tile([C, 8], f32, tag="ssqs", name="ssqs")
                nc.vector.tensor_copy(ssqs[:], pss[:])
                invc = spool.tile([C, 8], f32, tag="invc", name="invc")
                nc.gpsimd.tensor_tensor(invc[:], ssqs[:], neghalf, Alu.pow)
                state[("f2", t)] = (ssqs, invc)

            def emit_f1(t):
                """Logits + transpose matmuls and the xT eviction of chunk t
                (one chunk ahead)."""
                if t >= NT:
                    return
                xb = state["dma"].pop(t)
                psl = pL.tile([C, TPC * K], f32, tag="L", name="psl")
                pst = pT.tile([C, TPC * 128], f32, tag="T", name="pst")
                for j in range(TPC):
                    xt = xb[:, j * 128:(j + 1) * 128]
                    nc.tensor.matmul(psl[:, j * K:(j + 1) * K], xt, wTb,
                                     start=True, stop=True)
                    nc.tensor.matmul(pst[:, j * 128:(j + 1) * 128], xt,
                                     identb, start=True, stop=True)
                # evict xT -> [x | n] slabs (bf16) on ACT.  Two copies: a
                # PSUM AP may not cross the 2 KB bank boundary.
                xTs = wpool.tile([C, TPC * 129], bf16, tag="xTs", name="xTs")
                xv = xTs[:].rearrange("p (t q) -> p t q", q=129)
                h = TPC // 2
                pv_ = pst[:].rearrange("p (t q) -> p t q", q=128)
                nc.scalar.activation(xv[:, 0:h, 0:128], pv_[:, 0:h, :],
                                     Act.Copy)
                nc.scalar.activation(xv[:, h:TPC, 0:128], pv_[:, h:TPC, :],
                                     Act.Copy)
                state[("f1", t)] = (psl, xTs, xv)

            def emit_back(t):
                """Chain tail of chunk t: softmax, weights, vlad."""
                img, ch = divmod(t, NCH)
                ssqs, invc = state.pop(("f2", t))
                psl, xTs, xv = state.pop(("f1", t))
                psumL = psl[:].rearrange("p (t k) -> p t k", k=K)

                l3 = lambda tl, q: tl[:].rearrange("p (t k) -> p t k", k=q)
                # lu = raw * inv_n
                lu = wpool.tile([C, TPC * K], f32, tag="lu")
                nc.vector.tensor_tensor(
                    l3(lu, K), psumL, invc[:].broadcast_to([C, TPC, K]),
                    Alu.mult)
                # ll = lu + b  (gpsimd)
                ll = wpool.tile([C, TPC * K], f32, tag="ll")
                nc.gpsimd.tensor_tensor(ll[:], lu[:], b8h, Alu.add)
                # negm = -max_k ll
                negm = spool.tile([C, 8], f32, tag="negm")
                nc.vector.tensor_reduce(negm[:], l3(ll, K), axis=AxX,
                                        op=Alu.max, negate=True)
                # dd = ll - m
                dd = wpool.tile([C, TPC * K], f32, tag="dd")
                nc.vector.tensor_tensor(
                    l3(dd, K), l3(ll, K), negm[:].broadcast_to([C, TPC, K]),
                    Alu.add)
                # ee = exp(dd) (bf16)
                ee = wpool.tile([C, TPC * K], bf16, tag="ee")
                nc.scalar.activation(ee[:], dd[:], Act.Exp)
                scol = spool.tile([C, 8], f32, tag="scol")
                nc.vector.tensor_reduce(scol[:], l3(ee, K), axis=AxX,
                                        op=Alu.add)
                # r = inv_n / sumexp (bf16)
                gcol = spool.tile([C, 8], f32, tag="gcol")
                nc.vector.reciprocal(gcol[:], scol[:])
                rcol = spool.tile([C, 8], bf16, tag="rcol")
                nc.vector.tensor_tensor(rcol[:], invc[:], gcol[:], Alu.mult)
                # aa = ee * r  (gpsimd)
                aa = wpool.tile([C, TPC * K], bf16, tag="aa")
                nc.gpsimd.tensor_tensor(
                    l3(aa, K), l3(ee, K), rcol[:].broadcast_to([C, TPC, K]),
                    Alu.mult)
                # n = ssq * inv_n into the 129th column (small, on DVE)
                svec = spool.tile([C, 8], f32, tag="svec")
                nc.vector.tensor_tensor(svec[:], ssqs[:], invc[:], Alu.mult)
                s3 = lambda tl: tl[:].rearrange("p (t o) -> p t o", o=1)
                nc.vector.tensor_copy(xv[:, :, 128:129], s3(svec))

                if ch == 0:
                    state["psV"] = pV.tile([C, 512], f32, tag="psV",
                                           name="psv")
                psv = state["psV"]
                for j in range(TPC):
                    nc.tensor.matmul(psv[0:KE, 0:129],
                                     aa[:, j * K:j * K + KE],
                                     xTs[:, j * 129:(j + 1) * 129],
                                     start=(ch == 0 and j == 0),
                                     stop=(ch == NCH - 1 and j == TPC - 1))
                if ch == NCH - 1:
                    emit_tail(img, psv)

            def emit_tail(img, psv):
                # vk = term1 - s*cen  [56, 128] fp32
                negs = spool.tile([KE, 1], f32, tag="negs")
                nc.vector.tensor_scalar_mul(negs[:], psv[0:KE, 128:129], -1.0)
                vk = fpool.tile([KE, C], f32, tag="vk")
                nc.vector.scalar_tensor_tensor(vk[:], cen, negs[:],
                                               psv[0:KE, 0:C],
                                               Alu.mult, Alu.add)
                # transpose -> [c, k] into the same psV bank (cols 160:216)
                nc.tensor.matmul(psv[:, 160:160 + KE], vk[:],
                                 identf[0:KE, 0:KE],
                                 is_transpose=True, start=True, stop=True)
                vkT = psv[:, 160:160 + KE]
                trash = fpool.tile([C, KE], bf16, tag="trash")
                ssqk = spool.tile([C, 1], f32, tag="ssqk")
                nc.scalar.activation(trash[:], vkT, Act.Square,
                                     accum_out=ssqk[:])
                ssqc = spool.tile([C, 1], f32, tag="ssqc")
                nc.vector.tensor_scalar_max(ssqc[:], ssqk[:], 1e-24)
                invk = spool.tile([C, 1], f32, tag="invk")
                nc.gpsimd.tensor_tensor(invk[:], ssqc[:], neghalf[:, 0:1],
                                        Alu.pow)
                # q = ssqk * invk^2  (per-partition contribution to ||.||_F^2)
                iv2 = spool.tile([C, 1], f32, tag="iv2")
                nc.vector.tensor_tensor(iv2[:], invk[:], invk[:], Alu.mult)
                qv = spool.tile([C, 1], f32, tag="qv")
                nc.vector.tensor_tensor(qv[:], ssqc[:], iv2[:], Alu.mult)
                # tot = sum_c q  via PE (fp32 tiny)
                nc.tensor.matmul(psv[0:1, 216:217], qv[:], onesf,
                                 start=True, stop=True)
                tot = spool.tile([1, 1], f32, tag="tot")
                nc.vector.tensor_scalar_max(tot[:], psv[0:1, 216:217], 1e-24)
                fv = spool.tile([1, 1], f32, tag="fv")
                nc.gpsimd.tensor_tensor(fv[:], tot[:], neghalf[0:1, 0:1],
                                        Alu.pow)
                # broadcast fv to all partitions, comb = invk * fv
                nc.tensor.matmul(psv[:, 218:219], onesrow, fv[:],
                                 start=True, stop=True)
                comb = spool.tile([C, 1], f32, tag="comb")
                nc.vector.tensor_tensor(comb[:], invk[:], psv[:, 218:219],
                                        Alu.mult)
                obT = fpool.tile([C, KE], f32, tag="obT")
                nc.vector.tensor_scalar(obT[:], vkT, comb[:], None, Alu.mult)
                # transpose back -> [k, c] (cols 256:384), evict, DMA out
                nc.tensor.matmul(psv[0:KE, 256:384], obT[:], identf,
                                 is_transpose=True, start=True, stop=True)
                ob = fpool.tile([KE, C], f32, tag="ob")
                nc.scalar.activation(ob[:], psv[0:KE, 256:384], Act.Copy)
                nc.sync.dma_start(out_ext[img], ob[:])

            emit_f2(0)
            emit_f2(1)
            emit_f1(0)
            for t in range(NT):
                emit_f2(t + 2)
                emit_f1(t + 1)
                emit_back(t)

    nc.compile()
    return nc


def _get_nc():
    if "nc" not in _cache:
        _cache["nc"] = _build()
    return _cache["nc"]


def _make_in_maps(inputs):
    import ml_dtypes

    x = np.asarray(inputs["x"], dtype=np.float32)
    conv_w = np.asarray(inputs["conv_w"], dtype=np.float32)
    conv_b = np.asarray(inputs["conv_b"], dtype=np.float32)
    centroids = np.asarray(inputs["centroids"], dtype=np.float32)

    N = x.shape[0]
    n_cores = 8
    per = N // n_cores
    assert per == NIMG

    xb = x.reshape(N, C, P).astype(ml_dtypes.bfloat16)

    cstb = np.zeros((C, 193), dtype=ml_dtypes.bfloat16)
    cstb[:, 0:K] = conv_w.T.astype(ml_dtypes.bfloat16)
    cstb[:, K:K + C] = np.eye(C, dtype=np.float32)
    cstb[:, 192] = 1.0

    csth = np.tile(conv_b.astype(np.float32), TPC)[None, :].repeat(C, axis=0)
    csth = np.ascontiguousarray(csth)

    cstf = np.zeros((C, 400), dtype=np.float32)
    cstf[:, 0:C] = np.eye(C, dtype=np.float32)
    cstf[0:KE, C:C + C] = centroids[:KE]
    cstf[:, 256] = 1.0
    cstf[0, 258:386] = 1.0
    cstf[:, 392:400] = -0.5

    in_maps = []
    for i in range(n_cores):
        in_maps.append({
            "xb": np.ascontiguousarray(xb[i * per:(i + 1) * per]),
            "cstb": cstb,
            "csth": csth,
            "cstf": cstf,
        })
    return in_maps


def kernel(x, conv_w, conv_b, centroids):
    from concourse.bass_utils import run_bass_kernel_spmd

    in_maps = _make_in_maps(
        {"x": x, "conv_w": conv_w, "conv_b": conv_b, "centroids": centroids}
    )
    nc = _get_nc()
    res = run_bass_kernel_spmd(nc, in_maps, list(range(8)))
    outs = [np.asarray(r["out"]).reshape(NIMG, KE * C) for r in res.results]
    return np.concatenate(outs, axis=0)


if __name__ == "__main__":
    rng = np.random.default_rng(0)
    x = rng.standard_normal((32, C, 64, 64), dtype=np.float32)
    w = rng.standard_normal((K, C), dtype=np.float32)
    b = rng.standard_normal((K,), dtype=np.float32)
    c = rng.random((K, C), dtype=np.float32)
    out = kernel(x=x, conv_w=w, conv_b=b, centroids=c)
    print(out.shape, out.dtype)
